# revision 20
# baseline (speedup 1.0000x reference)
"""Fused LoRA-Linear (per-token adapter routing) for 8 TRN2 NeuronCores.

Strategy:
  - Shard tokens: 8192 -> 1024 per core. Replicate weight/adapters.
  - Stack the 8 rank-16 adapters into one 128-row block:
        A_cat [128, 4096],  B_cat.T [128, 4096]
    Prologue per core: a_allT = A_cat @ x_shard^T  -> [128, 1024]  (PE)
    then ams = a_allT * smask where smask[j,t] = scal[t]*(idx[t]==j//16).
  - All matmul operands are float32r (fp32 storage, reduced-precision
    multiply, fp32 PSUM): measured 227ns/512-row matmul sustained on HW.
    bf16 operands measure 259ns (P0 power downclock) -- slower despite
    half the DMA; mixed f32r*bf16 is rejected by the BIR verifier.
  - xT ships over DMA as bf16 and is upcast to resident f32r tiles by
    the (otherwise idle) Vector engine: halves the 16.8MB xT stream that
    made the n=0 phase DMA-bound.
  - DMA queue ops cost ~610ns regardless of size up to 256KB, so small
    loads are merged: A tiles in 4-k chunks, W tiles in 2-k pairs
    (288 -> 144 W queue ops).
  - n=0 sweep1: k-major m=0..5 + fused prologue (PSUM: 2+6 banks),
    streaming xT/A/W[0]. smask loads mid-sweep inside DMA slack.
  - n=0 sweep2: m=6,7 k-ordered [16..31, 0..15]: first half reuses the
    8 W pairs still resident in the 8-buf pool while the evicted
    k=0..15 pairs re-stream, hiding the reload entirely.
  - n=1..6: k-major, 8 PSUM banks; W[n] streams just-in-time. Each
    sweep pre-issues the NEXT n's first two W pairs near its end so the
    ~2us DMA-completion semaphore latency never stalls a sweep start.
    Per-m drains (DVE bias add) + out DMA overlap the next n's compute.
  - n=7 splits m=0..5 / m=6,7 like n=0 so the final drains stagger
    against real compute instead of trailing the kernel.
  - 3 fp32 warm-up matmuls on a zeroed scratch tile run during NEFF
    boot so the HAM clock-gate reaches 8/8 (2.4GHz) before real work.
"""

import numpy as np
import ml_dtypes

import concourse.bacc as bacc
import concourse.mybir as mybir
import concourse.tile as tile
from concourse.bass_utils import run_bass_kernel_spmd

SEQ, D_IN, D_OUT, RANK, N_ADAPTERS = 8192, 4096, 4096, 16, 8
N_CORES = 8
T = SEQ // N_CORES          # 1024 tokens per core
P = 128                     # partitions
FD = 512                    # matmul free dim (PSUM bank = 512 fp32)
KO = D_IN // P              # 32 contraction tiles
NT = D_OUT // FD            # 8 output column chunks
MT = T // P                 # 8 token tiles per core
J = N_ADAPTERS * RANK       # 128 stacked adapter rows
F32 = mybir.dt.float32
MMDT = mybir.dt.float32r    # matmul operand dtype (full-rate PE)
XBDT = mybir.dt.bfloat16    # xT DMA wire format (upcast on-chip)

_NC_CACHE = {}


def _build_nc():
    if "nc" in _NC_CACHE:
        return _NC_CACHE["nc"]
    nc = bacc.Bacc(None, target_bir_lowering=False, debug=False)
    xT = nc.dram_tensor("xT", [D_IN, T], XBDT, kind="ExternalInput")
    w = nc.dram_tensor("w", [NT, KO // 2, P, 2 * FD], MMDT, kind="ExternalInput")
    biasb = nc.dram_tensor("biasb", [NT, P, FD], F32, kind="ExternalInput")
    at = nc.dram_tensor("at", [KO // 2, P, 2 * J], MMDT, kind="ExternalInput")
    bt = nc.dram_tensor("bt", [NT, J, FD], MMDT, kind="ExternalInput")
    smask = nc.dram_tensor("smask", [J, T], F32, kind="ExternalInput")
    out = nc.dram_tensor("out", [T, D_OUT], F32, kind="ExternalOutput")

    with tile.TileContext(nc) as tc:
        with (
            tc.tile_pool(name="xtb", bufs=3) as xtb_pool,
            tc.tile_pool(name="xt", bufs=1) as xt_pool,
            tc.tile_pool(name="wp", bufs=8) as w_pool,
            tc.tile_pool(name="apool", bufs=3) as a_pool,
            tc.tile_pool(name="bp", bufs=2) as b_pool,
            tc.tile_pool(name="biasp", bufs=2) as bias_pool,
            tc.tile_pool(name="outp", bufs=5) as out_pool,
            tc.tile_pool(name="misc", bufs=1) as misc_pool,
            tc.tile_pool(name="psum", bufs=8, space="PSUM") as psum_pool,
        ):
            xT_v = xT[:].rearrange("(ko p) t -> ko p t", p=P)
            w_v = w[:]
            bias_v = biasb[:]
            at_v = at[:]
            bt_v = bt[:]
            out_v = out[:]
            smask_v = smask[:]

            smask_sb = misc_pool.tile([J, T], F32, tag="smask")
            ams = misc_pool.tile([J, T], MMDT, tag="ams")
            scratch = misc_pool.tile([P, FD], F32, tag="scratch")

            NCH = T // FD  # a_allT token chunks (2)
            xts = [None] * KO
            wts = {}       # (n, pair) -> W pair tile [P, 2*FD]
            a_chs = {}     # 2-k A chunks (256KB: per-DMA fixed-cost floor)
            b_sbs = {}
            bias_sbs = {}

            def load_b_bias(n):
                b_sbs[n] = b_pool.tile([J, FD], MMDT, tag="b", name=f"b{n}")
                nc.sync.dma_start(b_sbs[n][:], bt_v[n])
                bias_sbs[n] = bias_pool.tile(
                    [P, FD], F32, tag="bias", name=f"bias{n}"
                )
                nc.sync.dma_start(bias_sbs[n][:], bias_v[n])

            def load_w(n, i):
                wts[(n, i)] = w_pool.tile(
                    [P, 2 * FD], MMDT, tag="w", name=f"w{n}_{i}"
                )
                nc.sync.dma_start(wts[(n, i)][:], w_v[n, i])

            def w_slice(n, k):
                return wts[(n, k // 2)][:, (k % 2) * FD:(k % 2 + 1) * FD]

            def lora_drain(ps, n, m):
                nc.tensor.matmul(
                    ps[:], ams[:, m * P:(m + 1) * P], b_sbs[n][:],
                    start=False, stop=True,
                )
                o_sb = out_pool.tile([P, FD], F32, tag="o", name=f"o{n}_{m}")
                nc.vector.tensor_add(out=o_sb[:], in0=ps[:], in1=bias_sbs[n][:])
                nc.sync.dma_start(
                    out_v[m * P:(m + 1) * P, n * FD:(n + 1) * FD], o_sb[:]
                )

            def pre_issue_next(n, k):
                # near the end of n's sweep, start the next n's first two
                # W pairs (k=0..3) so the DMA-completion semaphore latency
                # is absorbed before the next sweep's first matmul.
                if k >= 28 and k % 2 == 0 and n + 1 < NT:
                    i = (k - 28) // 2
                    if (n + 1, i) not in wts:
                        load_w(n + 1, i)

            def tail_sweep(n, next_n=None):
                # m=6,7 second sweep: k-order [16..31, 0..15] reuses the 8
                # W pairs still resident in the 8-buf pool; the evicted
                # k<16 pairs re-stream during the first half, hiding the
                # reload behind the resident half's compute.
                pss = {
                    m: psum_pool.tile([P, FD], F32, tag="ps", name=f"ps_{n}b_{m}")
                    for m in (6, 7)
                }
                order = list(range(16, KO)) + list(range(16))
                for j, k in enumerate(order):
                    if j < 16 and j % 2 == 0:
                        load_w(n, j // 2)  # reload evicted pair
                    if next_n is not None and j >= 28 and j % 2 == 0:
                        load_w(next_n, (j - 28) // 2)
                    for m in (6, 7):
                        nc.tensor.matmul(
                            pss[m][:], xts[k][:, m * P:(m + 1) * P],
                            w_slice(n, k), start=(j == 0), stop=False,
                        )
                    if j == KO - 1:
                        for m in (6, 7):
                            lora_drain(pss[m], n, m)

            # ---- PE warm-up during NEFF boot + first DMAs ----
            nc.gpsimd.memset(scratch[:], 0)
            ps_warm = psum_pool.tile([P, FD], F32, tag="ps", name="ps_warm")
            for _ in range(3):
                # fp32 matmul = 4 cycles/row: ~1.7us of PE busy each at the
                # cold clock, enough to flip the HAM gate to 8/8 before the
                # first real matmul.
                nc.tensor.matmul(
                    ps_warm[:], scratch[:, :P], scratch[:],
                    start=True, stop=True,
                )

            # ---- n=0 sweep 1 (m=0..5, fused prologue, streams xT) ----
            psa = [
                psum_pool.tile([P, FD], F32, tag="ps", name=f"psa_{c}")
                for c in range(NCH)
            ]
            pss = {
                m: psum_pool.tile([P, FD], F32, tag="ps", name=f"ps_0_{m}")
                for m in range(6)
            }
            for k in range(KO):
                last_k = k == KO - 1
                xtb = xtb_pool.tile([P, T], XBDT, tag="xtb", name=f"xtb{k}")
                nc.sync.dma_start(xtb[:], xT_v[k])
                if k % 2 == 0:
                    for c in ([0, 1] if k == 0 else [k // 2 + 1]):
                        if c < KO // 2:
                            a_chs[c] = a_pool.tile(
                                [P, 2 * J], MMDT, tag="a", name=f"ach{c}"
                            )
                            nc.sync.dma_start(a_chs[c][:], at_v[c])
                a_sb = a_chs[k // 2][:, (k % 2) * J:(k % 2 + 1) * J]
                if k % 2 == 0:
                    load_w(0, k // 2)
                if k == 6:
                    load_b_bias(0)
                if k == 15:
                    nc.sync.dma_start(smask_sb[:, :FD], smask_v[:, :FD])
                if k == 17:
                    nc.sync.dma_start(smask_sb[:, FD:], smask_v[:, FD:])
                xts[k] = xt_pool.tile([P, T], MMDT, tag=f"xt{k}", name=f"xt{k}")
                nc.vector.tensor_copy(xts[k][:], xtb[:])
                for c in range(NCH):
                    nc.tensor.matmul(
                        psa[c][:], a_sb, xts[k][:, c * FD:(c + 1) * FD],
                        start=(k == 0), stop=last_k,
                    )
                for m in range(6):
                    nc.tensor.matmul(
                        pss[m][:], xts[k][:, m * P:(m + 1) * P], w_slice(0, k),
                        start=(k == 0), stop=False,
                    )
                if last_k:
                    for c in range(NCH):
                        nc.vector.tensor_mul(
                            out=ams[:, c * FD:(c + 1) * FD],
                            in0=psa[c][:],
                            in1=smask_sb[:, c * FD:(c + 1) * FD],
                        )
                    for m in range(6):
                        lora_drain(pss[m], 0, m)

            # ---- n=0 sweep 2 (m=6,7) ----
            tail_sweep(0, next_n=1)

            # ---- n=1..6: k-major, 8 banks, W streams just-in-time ----
            for n in range(1, NT - 1):
                load_b_bias(n)
                pss = {
                    m: psum_pool.tile([P, FD], F32, tag="ps", name=f"ps_{n}_{m}")
                    for m in range(MT)
                }
                for k in range(KO):
                    if k % 2 == 0 and (n, k // 2) not in wts:
                        load_w(n, k // 2)
                    pre_issue_next(n, k)
                    for m in range(MT):
                        nc.tensor.matmul(
                            pss[m][:], xts[k][:, m * P:(m + 1) * P],
                            w_slice(n, k), start=(k == 0), stop=False,
                        )
                    if k == KO - 1:
                        for m in range(MT):
                            lora_drain(pss[m], n, m)

            # ---- n=7: m=0..5 sweep then m=6,7 tail sweep ----
            n = NT - 1
            load_b_bias(n)
            pss = {
                m: psum_pool.tile([P, FD], F32, tag="ps", name=f"ps_{n}_{m}")
                for m in range(6)
            }
            for k in range(KO):
                if k % 2 == 0 and (n, k // 2) not in wts:
                    load_w(n, k // 2)
                for m in range(6):
                    nc.tensor.matmul(
                        pss[m][:], xts[k][:, m * P:(m + 1) * P], w_slice(n, k),
                        start=(k == 0), stop=False,
                    )
                if k == KO - 1:
                    for m in range(6):
                        lora_drain(pss[m], n, m)
            tail_sweep(n)

    nc.compile()
    _NC_CACHE["nc"] = nc
    return nc


def _prep_in_maps(x, weight, bias, A_buffer, B_buffer, scalings, token_indices):
    x = np.ascontiguousarray(np.asarray(x, np.float32))
    weight = np.asarray(weight, np.float32)
    bias = np.asarray(bias, np.float32)
    A_buffer = np.asarray(A_buffer, np.float32)
    B_buffer = np.asarray(B_buffer, np.float32)
    scalings = np.asarray(scalings, np.float32)
    token_indices = np.asarray(token_indices)

    xT_full = np.ascontiguousarray(x.T.astype(ml_dtypes.bfloat16))  # [D_IN, SEQ]
    # W pairs: w[n, i, p, j*FD+f] = weight[(2i+j)*128+p, n*512+f]
    w_t = np.ascontiguousarray(
        weight.reshape(KO // 2, 2, P, NT, FD).transpose(3, 0, 2, 1, 4)
        .reshape(NT, KO // 2, P, 2 * FD)
    )
    biasb = np.ascontiguousarray(
        np.broadcast_to(bias.reshape(NT, FD)[:, None, :], (NT, P, FD))
    )
    A_cat = A_buffer.reshape(J, D_IN)
    at = np.ascontiguousarray(
        A_cat.T.reshape(KO // 2, 2, P, J).transpose(0, 2, 1, 3)
        .reshape(KO // 2, P, 2 * J)
    )
    bt = np.ascontiguousarray(
        B_buffer.transpose(0, 2, 1).reshape(J, NT, FD).transpose(1, 0, 2)
    )  # [NT, J, FD]
    adapter_of_row = (np.arange(J) // RANK).astype(token_indices.dtype)
    smask_full = (
        (token_indices[None, :] == adapter_of_row[:, None]).astype(np.float32)
        * scalings[None, :]
    )  # [J, SEQ]

    in_maps = []
    for c in range(N_CORES):
        sl = slice(c * T, (c + 1) * T)
        in_maps.append({
            "xT": np.ascontiguousarray(xT_full[:, sl]),
            "w": w_t,
            "biasb": biasb,
            "at": at,
            "bt": bt,
            "smask": np.ascontiguousarray(smask_full[:, sl]),
        })
    return in_maps


def _run(inputs, trace=False):
    nc = _build_nc()
    in_maps = _prep_in_maps(**inputs)
    res = run_bass_kernel_spmd(
        nc, in_maps, core_ids=list(range(N_CORES)), trace=trace
    )
    out = np.concatenate([r["out"] for r in res.results], axis=0)
    return out, res


def kernel(**inputs) -> np.ndarray:
    out, _ = _run(inputs, trace=False)
    return out
